# revision 49
# baseline (speedup 1.0000x reference)
"""Multi-head attention Trainium2 Bass kernel.

Problem: B=2, T=2048, D=1024, H=16 heads, head dim K=64.
Sharding: 8 cores = 2 batches x 4 head-groups (4 heads each).
Each core computes q/k/v projections for its head slice, attention for its
4 heads, and a partial output projection; host sums partials over head
groups and adds the output bias.

Projections/logits run in float32r (TF32-like, full PE rate for moving
free dim >= 256); x is cast to bf16 for the PE transposes (1 cyc/row vs
2 for f32); the softmax P and V run in fp16.

Layout choices (per core):
  xT      [128p, ND, T]  x transposed, d on partitions (PE transpose of
                         the bf16-cast pages)
  qt/kt   [128p, 2, T]   Q^T/K^T: hk on partitions (head 2j at p 0:64 of
                         slab j, head 2j+1 at p 64:128) -> row-packed S
  v_aug   [128p, NT, 4, 65]  V natural (fp16) per k-tile/head + ones col
  S^T     [k=128, q]     logits transposed; exp'd to fp16 pp (ACT)
  PV      pp chunk [128k, 128q] is the STATIONARY operand, v_aug the
          moving one -> out is attention-NATURAL [128q, 65] with sumexp
          in col 64 (ones col of v_aug).  65-row moving stream per chunk
          instead of 512 -> PV stream time halves vs the S^T-moving form,
          and the softmax denominator becomes a per-partition scalar:
          normalize is reciprocal([128,1]) + tensor_scalar_mul.
  attnT   [128p, 2, T]   normalized attention transposed back via PE
                         (pair-stacked) = stationary layout for out proj
"""

from contextlib import ExitStack

import numpy as np

import concourse.bass as bass
import concourse.tile as tile
from concourse import bacc, mybir
from concourse.masks import make_identity

F32 = mybir.dt.float32
F32R = mybir.dt.float32r
BF16 = mybir.dt.bfloat16
FP16 = mybir.dt.float16

# problem config (hardcoded per contest rules)
B, T, D = 2, 2048, 1024
H, HK = 16, 64          # total heads, head dim
NCORES = 8
HPC = H // (NCORES // B)   # heads per core = 4
DH = HPC * HK              # per-core hk slice width = 256
QC = 512                   # free-dim chunk for matmuls


def body(tc, outs, ins, cfg, reps=1):
    nc = tc.nc
    T_, D_, DH_ = cfg["T"], cfg["D"], cfg["DH"]
    QC_ = cfg["QC"]
    NT = T_ // 128          # token tiles
    ND = D_ // 128          # d tiles
    NQ = T_ // QC_          # token chunks for matmul free dim
    NHT = DH_ // 128        # hk partition slabs (= head pairs)
    NKT = NT                # k-token tiles in attention

    x, wq, wk, wv, wo, bq, bk, bv = (
        ins["x"], ins["wq"], ins["wk"], ins["wv"], ins["wo"],
        ins["bq"], ins["bk"], ins["bv"],
    )
    out = outs["out"]

    with ExitStack() as ctx:
        # ---- persistent SBUF tensors ----
        xT = nc.alloc_sbuf_tensor("xT", [128, ND, T_], F32R).ap()
        qt = nc.alloc_sbuf_tensor("qt", [128, NHT, T_], F32R).ap()
        kt = nc.alloc_sbuf_tensor("kt", [128, NHT, T_], F32R).ap()
        v_aug = nc.alloc_sbuf_tensor("v_aug", [128, NKT, HPC, HK + 1], FP16).ap()
        wo32r = nc.alloc_sbuf_tensor("wo32r", [128, NHT, D_], F32R).ap()
        ident = nc.alloc_sbuf_tensor("ident", [128, 128], F32).ap()
        ident_bf = nc.alloc_sbuf_tensor("ident_bf", [128, 128], BF16).ap()
        bq_sb = nc.alloc_sbuf_tensor("bq_sb", [128, NHT], F32).ap()
        bk_sb = nc.alloc_sbuf_tensor("bk_sb", [128, NHT], F32).ap()
        bv_row = nc.alloc_sbuf_tensor("bv_row", [1, DH_], F32).ap()
        bv_bc = nc.alloc_sbuf_tensor("bv_bc", [128, DH_], F32).ap()

        # ---- pools ----
        pg4k = ctx.enter_context(tc.tile_pool(name="pg4k", bufs=3))
        wraw = ctx.enter_context(tc.tile_pool(name="wraw", bufs=1))
        w32r = ctx.enter_context(tc.tile_pool(name="w32r", bufs=3))
        ptil = ctx.enter_context(tc.tile_pool(name="ptil", bufs=4))
        atp = ctx.enter_context(tc.tile_pool(name="atp", bufs=2))
        outp = ctx.enter_context(tc.tile_pool(name="outp", bufs=2))
        natp = ctx.enter_context(tc.tile_pool(name="natp", bufs=4))
        rzp = ctx.enter_context(tc.tile_pool(name="rzp", bufs=8))
        psA = ctx.enter_context(tc.tile_pool(name="psA", bufs=2, space="PSUM"))
        psS = ctx.enter_context(tc.tile_pool(name="psS", bufs=2, space="PSUM"))
        psO = ctx.enter_context(tc.tile_pool(name="psO", bufs=2, space="PSUM"))

        make_identity(nc, ident)
        nc.vector.tensor_copy(ident_bf[:, :], ident[:, :])

        # biases (issued from gpsimd so the ACT engine stays free for exp)
        for t in range(NHT):
            nc.gpsimd.dma_start(bq_sb[:, t:t + 1],
                                bq[t * 128:(t + 1) * 128].unsqueeze(1))
            nc.gpsimd.dma_start(bk_sb[:, t:t + 1],
                                bk[t * 128:(t + 1) * 128].unsqueeze(1))
        nc.gpsimd.dma_start(bv_row[:, :], bv.unsqueeze(0))
        nc.gpsimd.partition_broadcast(bv_bc[:, :], bv_row[:, :])
        ones_f32 = nc.alloc_sbuf_tensor("ones_f32", [128, NKT * HPC], F32).ap()

        for _rep in range(reps):
            # ---- phase 0: weights load + round to f32r ----
            def load_w32r(w_dram, tag):
                wr = wraw.tile([128, ND, DH_], F32, tag="wraw")
                for dt in range(ND):
                    nc.gpsimd.dma_start(wr[:, dt, :], w_dram[dt * 128:(dt + 1) * 128, :])
                w2 = w32r.tile([128, ND, DH_], F32R, tag="w32r")
                nc.vector.tensor_copy(w2[:, :, :], wr[:, :, :])
                return w2

            wv2 = load_w32r(wv, "wv")
            # wk/wq are hoisted before the page loop so their DVE casts sit
            # ahead of the page copies in the queue -- the k/q projections
            # otherwise stall on them right after phase 1
            wk2 = load_w32r(wk, "wk")
            wq2 = load_w32r(wq, "wq")
            nc.vector.memset(ones_f32[:, :], 1.0)
            nc.vector.tensor_copy(
                v_aug[:, :, :, HK:HK + 1],
                ones_f32.rearrange("p (n h) -> p n h", h=HPC).unsqueeze(3),
            )

            # ---- phase 1: per token page: load x, cast bf16, transpose, V(n)
            # bf16 transposes run the PE at 1 cyc/row (vs 2 for f32)
            TG = 4
            for n in range(NT):
                xpg = pg4k.tile([128, D_], F32, tag="pg")
                nc.sync.dma_start(xpg[:, :], x[n * 128:(n + 1) * 128, :])
                xbf = pg4k.tile([128, D_], BF16, tag="xbf")
                # cast on ACT: idle until the first exp in a single-shot
                # run, and DVE (copies + v_aug adds) otherwise paces phase 1
                nc.scalar.copy(xbf[:, :], xpg[:, :])
                for dg in range(ND // TG):
                    tp = psA.tile([128, TG, 128], BF16, tag="psA")
                    for i in range(TG):
                        dt = dg * TG + i
                        nc.tensor.transpose(tp[:, i, :],
                                            xbf[:, dt * 128:(dt + 1) * 128],
                                            ident_bf[:, :])
                    dst = xT[:, dg * TG:(dg + 1) * TG, n * 128:(n + 1) * 128]
                    nc.vector.tensor_copy(dst, tp[:, :, :])
                # V(n): needs all d-tiles of this token page
                ps = psA.tile([128, DH_], F32, tag="psA")
                for dt in range(ND):
                    nc.tensor.matmul(
                        ps[:, :],
                        xT[:, dt, n * 128:(n + 1) * 128],
                        wv2[:, dt, :],
                        start=(dt == 0), stop=(dt == ND - 1),
                    )
                # v_aug[:, n, h, 0:HK] = ps[:, h*HK:(h+1)*HK] + bv, all h at once
                nc.vector.tensor_add(
                    v_aug[:, n, :, 0:HK],
                    ps.rearrange("p (h e) -> p h e", h=HPC),
                    bv_bc.rearrange("p (h e) -> p h e", h=HPC),
                )

            wor = wraw.tile([128, NHT, D_], F32, tag="wraw")
            for ht in range(NHT):
                nc.gpsimd.dma_start(wor[:, ht, :], wo[ht * 128:(ht + 1) * 128, :])
            nc.vector.tensor_copy(wo32r[:, :, :], wor[:, :, :])

            # ---- phase 2: q/k projections (k first: attention's blocker) ----
            def project_slab(w2, bias_sb, dst, ht):
                for qcc in range(NQ):
                    ps = psA.tile([128, QC_], F32, tag="psA")
                    for dt in range(ND):
                        nc.tensor.matmul(
                            ps[:, :],
                            w2[:, dt, ht * 128:(ht + 1) * 128],
                            xT[:, dt, qcc * QC_:(qcc + 1) * QC_],
                            start=(dt == 0), stop=(dt == ND - 1),
                        )
                    nc.vector.tensor_scalar_add(
                        dst[:, ht, qcc * QC_:(qcc + 1) * QC_], ps[:, :],
                        bias_sb[:, ht:ht + 1],
                    )

            project_slab(wk2, bk_sb, kt, 0)
            project_slab(wk2, bk_sb, kt, 1)
            project_slab(wq2, bq_sb, qt, 0)
            project_slab(wq2, bq_sb, qt, 1)

            # ---- phase 3: attention, qc-outer so projection pipelines ----
            NOC = D_ // QC_
            NPQ = QC_ // 128  # token tiles per qc chunk
            for qcc in range(NQ):
                qsl = slice(qcc * QC_, (qcc + 1) * QC_)
                a_qc = atp.tile([128, NHT, QC_], F32R, tag="atp")
                for hp in range(NHT):
                    # 8 accumulators [128q, 65] = (head, q-tile): 4 per
                    # head packed as manual 65-col slices of a 1-bank PSUM
                    # tile; two tiles (one per head) rotate independently in
                    # the pool so the next head-pair's PV can start as soon
                    # as THIS head's normalize reads are done
                    po0 = psO.tile([128, 512], F32, tag="psO", name="po0")
                    po1 = psO.tile([128, 512], F32, tag="psO", name="po1")

                    def acc_ap(hl, npq):
                        po = po0 if hl == 0 else po1
                        return po[:, npq * 65:npq * 65 + HK + 1]

                    def emit_s_exp(ktt):
                        ksl = slice(ktt * 128, (ktt + 1) * 128)
                        sp = psS.tile([128, 2 * QC_], F32, tag="psS")
                        nc.tensor.matmul(sp[:, 0:QC_], kt[0:64, hp, ksl],
                                         qt[0:64, hp, qsl],
                                         start=True, stop=True, tile_position=(0, 0))
                        nc.tensor.matmul(sp[:, QC_:2 * QC_], kt[64:128, hp, ksl],
                                         qt[64:128, hp, qsl],
                                         start=True, stop=True, tile_position=(64, 0))
                        pp = ptil.tile([128, 2 * QC_], FP16, tag="ptil")
                        nc.scalar.activation(pp[:, :], sp[:, :],
                                             mybir.ActivationFunctionType.Exp,
                                             scale=float(1.0 / np.sqrt(HK)))
                        return pp

                    def emit_pv(ktt, pp):
                        # pp chunk stationary, V moving: out accumulates the
                        # attention-natural [q, v-dims + sumexp] per head.
                        # PSUM start=True zeroes the WHOLE 2KB bank (zero
                        # region), so only the first slice written in each
                        # bank (npq == 0) carries it; the rest accumulate
                        # onto the already-zeroed bank.
                        for hl in range(2):
                            for npq in range(NPQ):
                                nc.tensor.matmul(
                                    acc_ap(hl, npq),
                                    pp[:, hl * QC_ + npq * 128:
                                       hl * QC_ + (npq + 1) * 128],
                                    v_aug[:, ktt, 2 * hp + hl, :],
                                    start=(ktt == 0) and npq == 0,
                                    stop=(ktt == NKT - 1),
                                    skip_group_check=True,
                                )

                    # two-stage software pipeline: S(kt+2) issues before
                    # PV(kt), so the PE has QK work queued while the first
                    # PV of this head-pair waits for the accumulator tile
                    pps = [emit_s_exp(0), emit_s_exp(1)]
                    for ktt in range(2, NKT):
                        pps.append(emit_s_exp(ktt))
                        emit_pv(ktt - 2, pps[ktt - 2])
                    emit_pv(NKT - 2, pps[NKT - 2])
                    emit_pv(NKT - 1, pps[NKT - 1])
                    # normalize per (head, q-tile): denominator is col HK ->
                    # per-partition scalar; then PE-transpose the head pair
                    # back to the attnT layout for the output projection
                    nats = []
                    for npq in range(NPQ):
                        nat = natp.tile([128, 2 * HK], BF16, tag="natp")
                        for hl in range(2):
                            acc = acc_ap(hl, npq)
                            rz = rzp.tile([128, 1], F32, tag="rzp")
                            nc.vector.reciprocal(rz[:, :], acc[:, HK:HK + 1])
                            nc.vector.tensor_scalar_mul(
                                nat[:, hl * HK:(hl + 1) * HK],
                                acc[:, 0:HK], rz[:, :])
                        nats.append(nat)
                    tpn = psA.tile([128, NPQ, 128], BF16, tag="psA")
                    for npq in range(NPQ):
                        nc.tensor.transpose(tpn[:, npq, :], nats[npq][:, :],
                                            ident_bf[:, :])
                    nc.vector.tensor_copy(
                        a_qc[:, hp, :],
                        tpn.rearrange("p a b -> p (a b)"))

                # ---- output projection for this qc's token tiles ----
                for npq in range(NPQ):
                    n = qcc * NPQ + npq
                    for oc in range(NOC):
                        ps = psA.tile([128, QC_], F32, tag="psA")
                        for hp in range(NHT):
                            nc.tensor.matmul(
                                ps[:, :],
                                a_qc[:, hp, npq * 128:(npq + 1) * 128],
                                wo32r[:, hp, oc * QC_:(oc + 1) * QC_],
                                start=(hp == 0), stop=(hp == NHT - 1),
                            )
                        ot = outp.tile([128, QC_], F32, tag="outp")
                        nc.vector.tensor_copy(ot[:, :], ps[:, :])
                        nc.sync.dma_start(
                            out[n * 128:(n + 1) * 128, oc * QC_:(oc + 1) * QC_],
                            ot[:, :],
                        )


def build(cfg, reps=1):
    nc = bacc.Bacc("TRN2", target_bir_lowering=False, debug=False,
                   num_devices=NCORES)
    T_, D_, DH_ = cfg["T"], cfg["D"], cfg["DH"]
    ins = {
        "x": nc.dram_tensor("x", [T_, D_], F32, kind="ExternalInput").ap(),
        "wq": nc.dram_tensor("wq", [D_, DH_], F32, kind="ExternalInput").ap(),
        "wk": nc.dram_tensor("wk", [D_, DH_], F32, kind="ExternalInput").ap(),
        "wv": nc.dram_tensor("wv", [D_, DH_], F32, kind="ExternalInput").ap(),
        "wo": nc.dram_tensor("wo", [DH_, D_], F32, kind="ExternalInput").ap(),
        "bq": nc.dram_tensor("bq", [DH_], F32, kind="ExternalInput").ap(),
        "bk": nc.dram_tensor("bk", [DH_], F32, kind="ExternalInput").ap(),
        "bv": nc.dram_tensor("bv", [DH_], F32, kind="ExternalInput").ap(),
    }
    outs = {
        "out": nc.dram_tensor("out", [T_, D_], F32, kind="ExternalOutput").ap(),
    }
    with tile.TileContext(nc) as tc:
        body(tc, outs, ins, cfg, reps=reps)
    nc.compile()
    return nc


_NC_CACHE = {}


def _get_nc(reps=1):
    key = (T, D, DH, reps)
    if key not in _NC_CACHE:
        _NC_CACHE[key] = build({"T": T, "D": D, "DH": DH, "QC": QC}, reps=reps)
    return _NC_CACHE[key]


def make_in_maps(x_q, Wq, bq, Wk, bk, Wv, bv, Wo, bo):
    in_maps = []
    for c in range(NCORES):
        b, hg = divmod(c, NCORES // B)
        sl = slice(hg * DH, (hg + 1) * DH)
        in_maps.append({
            "x": np.ascontiguousarray(x_q[b], dtype=np.float32),
            "wq": np.ascontiguousarray(Wq[:, sl], dtype=np.float32),
            "wk": np.ascontiguousarray(Wk[:, sl], dtype=np.float32),
            "wv": np.ascontiguousarray(Wv[:, sl], dtype=np.float32),
            "wo": np.ascontiguousarray(Wo[sl, :], dtype=np.float32),
            "bq": np.ascontiguousarray(bq[sl], dtype=np.float32),
            "bk": np.ascontiguousarray(bk[sl], dtype=np.float32),
            "bv": np.ascontiguousarray(bv[sl], dtype=np.float32),
        })
    return in_maps


def gather(results, bo):
    ngrp = NCORES // B
    out = np.empty((B, T, D), dtype=np.float32)
    for b in range(B):
        acc = results[b * ngrp]["out"].astype(np.float32).copy()
        for hg in range(1, ngrp):
            acc += results[b * ngrp + hg]["out"]
        out[b] = acc + np.asarray(bo, dtype=np.float32)[None, :]
    return out


def kernel(x_q, Wq, bq, Wk, bk, Wv, bv, Wo, bo, _spmd_kwargs=None, _reps=1):
    from concourse.bass_utils import run_bass_kernel_spmd

    nc = _get_nc(reps=_reps)
    in_maps = make_in_maps(x_q, Wq, bq, Wk, bk, Wv, bv, Wo, bo)
    kw = _spmd_kwargs or {}
    res = run_bass_kernel_spmd(nc, in_maps, core_ids=list(range(NCORES)), **kw)
    out = gather(res.results, bo)
    kernel.last_results = res
    return out



# revision 50
# speedup vs baseline: 3.1094x; 3.1094x over previous
"""Multi-head attention Trainium2 Bass kernel.

Problem: B=2, T=2048, D=1024, H=16 heads, head dim K=64.
Sharding: 8 cores = 2 batches x 4 head-groups (4 heads each).
Each core computes q/k/v projections for its head slice, attention for its
4 heads, and a partial output projection; host sums partials over head
groups and adds the output bias.

Projections/logits run in float32r (TF32-like, full PE rate for moving
free dim >= 256); x is cast to bf16 for the PE transposes (1 cyc/row vs
2 for f32); the softmax P and V run in fp16.

Layout choices (per core):
  xT      [128p, ND, T]  x transposed, d on partitions (PE transpose of
                         the bf16-cast pages)
  qt/kt   [128p, 2, T]   Q^T/K^T: hk on partitions (head 2j at p 0:64 of
                         slab j, head 2j+1 at p 64:128) -> row-packed S
  v_aug   [128p, NT, 4, 65]  V natural (fp16) per k-tile/head + ones col
  S^T     [k=128, q]     logits transposed; exp'd to fp16 pp (ACT)
  PV      pp chunk [128k, 128q] is the STATIONARY operand, v_aug the
          moving one -> out is attention-NATURAL [128q, 65] with sumexp
          in col 64 (ones col of v_aug).  65-row moving stream per chunk
          instead of 512 -> PV stream time halves vs the S^T-moving form,
          and the softmax denominator becomes a per-partition scalar:
          normalize is reciprocal([128,1]) + tensor_scalar_mul.
  attnT   [128p, 2, T]   normalized attention transposed back via PE
                         (pair-stacked) = stationary layout for out proj
"""

from contextlib import ExitStack

import numpy as np

import concourse.bass as bass
import concourse.tile as tile
from concourse import bacc, mybir
from concourse.masks import make_identity

F32 = mybir.dt.float32
F32R = mybir.dt.float32r
BF16 = mybir.dt.bfloat16
FP16 = mybir.dt.float16

# problem config (hardcoded per contest rules)
B, T, D = 2, 2048, 1024
H, HK = 16, 64          # total heads, head dim
NCORES = 8
HPC = H // (NCORES // B)   # heads per core = 4
DH = HPC * HK              # per-core hk slice width = 256
QC = 512                   # free-dim chunk for matmuls


def body(tc, outs, ins, cfg, reps=1):
    nc = tc.nc
    T_, D_, DH_ = cfg["T"], cfg["D"], cfg["DH"]
    QC_ = cfg["QC"]
    NT = T_ // 128          # token tiles
    ND = D_ // 128          # d tiles
    NQ = T_ // QC_          # token chunks for matmul free dim
    NHT = DH_ // 128        # hk partition slabs (= head pairs)
    NKT = NT                # k-token tiles in attention

    x, wq, wk, wv, wo, bq, bk, bv = (
        ins["x"], ins["wq"], ins["wk"], ins["wv"], ins["wo"],
        ins["bq"], ins["bk"], ins["bv"],
    )
    out = outs["out"]

    with ExitStack() as ctx:
        # ---- persistent SBUF tensors ----
        xT = nc.alloc_sbuf_tensor("xT", [128, ND, T_], F32R).ap()
        qt = nc.alloc_sbuf_tensor("qt", [128, NHT, T_], F32R).ap()
        kt = nc.alloc_sbuf_tensor("kt", [128, NHT, T_], F32R).ap()
        v_aug = nc.alloc_sbuf_tensor("v_aug", [128, NKT, HPC, HK + 1], FP16).ap()
        wo32r = nc.alloc_sbuf_tensor("wo32r", [128, NHT, D_], F32R).ap()
        ident = nc.alloc_sbuf_tensor("ident", [128, 128], F32).ap()
        ident_bf = nc.alloc_sbuf_tensor("ident_bf", [128, 128], BF16).ap()
        bq_sb = nc.alloc_sbuf_tensor("bq_sb", [128, NHT], F32).ap()
        bk_sb = nc.alloc_sbuf_tensor("bk_sb", [128, NHT], F32).ap()
        bv_row = nc.alloc_sbuf_tensor("bv_row", [1, DH_], F32).ap()
        bv_bc = nc.alloc_sbuf_tensor("bv_bc", [128, DH_], F32).ap()

        # ---- pools ----
        pg4k = ctx.enter_context(tc.tile_pool(name="pg4k", bufs=3))
        wraw = ctx.enter_context(tc.tile_pool(name="wraw", bufs=1))
        w32r = ctx.enter_context(tc.tile_pool(name="w32r", bufs=3))
        ptil = ctx.enter_context(tc.tile_pool(name="ptil", bufs=4))
        atp = ctx.enter_context(tc.tile_pool(name="atp", bufs=2))
        outp = ctx.enter_context(tc.tile_pool(name="outp", bufs=2))
        natp = ctx.enter_context(tc.tile_pool(name="natp", bufs=4))
        rzp = ctx.enter_context(tc.tile_pool(name="rzp", bufs=8))
        psA = ctx.enter_context(tc.tile_pool(name="psA", bufs=2, space="PSUM"))
        psS = ctx.enter_context(tc.tile_pool(name="psS", bufs=2, space="PSUM"))
        psO = ctx.enter_context(tc.tile_pool(name="psO", bufs=2, space="PSUM"))

        make_identity(nc, ident)
        nc.vector.tensor_copy(ident_bf[:, :], ident[:, :])

        # biases (issued from gpsimd so the ACT engine stays free for exp)
        for t in range(NHT):
            nc.gpsimd.dma_start(bq_sb[:, t:t + 1],
                                bq[t * 128:(t + 1) * 128].unsqueeze(1))
            nc.gpsimd.dma_start(bk_sb[:, t:t + 1],
                                bk[t * 128:(t + 1) * 128].unsqueeze(1))
        nc.gpsimd.dma_start(bv_row[:, :], bv.unsqueeze(0))
        nc.gpsimd.partition_broadcast(bv_bc[:, :], bv_row[:, :])
        ones_f32 = nc.alloc_sbuf_tensor("ones_f32", [128, NKT * HPC], F32).ap()

        for _rep in range(reps):
            # ---- phase 0: weights load + round to f32r ----
            def load_w32r(w_dram, tag):
                wr = wraw.tile([128, ND, DH_], F32, tag="wraw")
                for dt in range(ND):
                    nc.gpsimd.dma_start(wr[:, dt, :], w_dram[dt * 128:(dt + 1) * 128, :])
                w2 = w32r.tile([128, ND, DH_], F32R, tag="w32r")
                nc.vector.tensor_copy(w2[:, :, :], wr[:, :, :])
                return w2

            wv2 = load_w32r(wv, "wv")
            # wk/wq are hoisted before the page loop so their DVE casts sit
            # ahead of the page copies in the queue -- the k/q projections
            # otherwise stall on them right after phase 1
            wk2 = load_w32r(wk, "wk")
            wq2 = load_w32r(wq, "wq")
            nc.vector.memset(ones_f32[:, :], 1.0)
            nc.vector.tensor_copy(
                v_aug[:, :, :, HK:HK + 1],
                ones_f32.rearrange("p (n h) -> p n h", h=HPC).unsqueeze(3),
            )

            # ---- phase 1: per token page: load x, cast bf16, transpose, V(n)
            # bf16 transposes run the PE at 1 cyc/row (vs 2 for f32)
            TG = 4
            for n in range(NT):
                xpg = pg4k.tile([128, D_], F32, tag="pg")
                nc.sync.dma_start(xpg[:, :], x[n * 128:(n + 1) * 128, :])
                xbf = pg4k.tile([128, D_], BF16, tag="xbf")
                # cast on ACT: idle until the first exp in a single-shot
                # run, and DVE (copies + v_aug adds) otherwise paces phase 1
                nc.scalar.copy(xbf[:, :], xpg[:, :])
                for dg in range(ND // TG):
                    tp = psA.tile([128, TG, 128], BF16, tag="psA")
                    for i in range(TG):
                        dt = dg * TG + i
                        nc.tensor.transpose(tp[:, i, :],
                                            xbf[:, dt * 128:(dt + 1) * 128],
                                            ident_bf[:, :])
                    dst = xT[:, dg * TG:(dg + 1) * TG, n * 128:(n + 1) * 128]
                    nc.vector.tensor_copy(dst, tp[:, :, :])
                # V(n): needs all d-tiles of this token page
                ps = psA.tile([128, DH_], F32, tag="psA")
                for dt in range(ND):
                    nc.tensor.matmul(
                        ps[:, :],
                        xT[:, dt, n * 128:(n + 1) * 128],
                        wv2[:, dt, :],
                        start=(dt == 0), stop=(dt == ND - 1),
                    )
                # v_aug[:, n, h, 0:HK] = ps[:, h*HK:(h+1)*HK] + bv, all h at once
                nc.vector.tensor_add(
                    v_aug[:, n, :, 0:HK],
                    ps.rearrange("p (h e) -> p h e", h=HPC),
                    bv_bc.rearrange("p (h e) -> p h e", h=HPC),
                )

            wor = wraw.tile([128, NHT, D_], F32, tag="wraw")
            for ht in range(NHT):
                nc.gpsimd.dma_start(wor[:, ht, :], wo[ht * 128:(ht + 1) * 128, :])
            nc.vector.tensor_copy(wo32r[:, :, :], wor[:, :, :])

            # ---- phase 2: q/k projections (k first: attention's blocker) ----
            def project_slab(w2, bias_sb, dst, ht):
                for qcc in range(NQ):
                    ps = psA.tile([128, QC_], F32, tag="psA")
                    for dt in range(ND):
                        nc.tensor.matmul(
                            ps[:, :],
                            w2[:, dt, ht * 128:(ht + 1) * 128],
                            xT[:, dt, qcc * QC_:(qcc + 1) * QC_],
                            start=(dt == 0), stop=(dt == ND - 1),
                        )
                    nc.vector.tensor_scalar_add(
                        dst[:, ht, qcc * QC_:(qcc + 1) * QC_], ps[:, :],
                        bias_sb[:, ht:ht + 1],
                    )

            project_slab(wk2, bk_sb, kt, 0)
            project_slab(wk2, bk_sb, kt, 1)
            project_slab(wq2, bq_sb, qt, 0)
            project_slab(wq2, bq_sb, qt, 1)

            # ---- phase 3: attention, qc-outer so projection pipelines ----
            NOC = D_ // QC_
            NPQ = QC_ // 128  # token tiles per qc chunk

            def s_exp(qc2, hp2, ktt):
                ksl = slice(ktt * 128, (ktt + 1) * 128)
                qs2 = slice(qc2 * QC_, (qc2 + 1) * QC_)
                sp = psS.tile([128, 2 * QC_], F32, tag="psS")
                nc.tensor.matmul(sp[:, 0:QC_], kt[0:64, hp2, ksl],
                                 qt[0:64, hp2, qs2],
                                 start=True, stop=True, tile_position=(0, 0))
                nc.tensor.matmul(sp[:, QC_:2 * QC_], kt[64:128, hp2, ksl],
                                 qt[64:128, hp2, qs2],
                                 start=True, stop=True, tile_position=(64, 0))
                pp = ptil.tile([128, 2 * QC_], FP16, tag="ptil")
                nc.scalar.activation(pp[:, :], sp[:, :],
                                     mybir.ActivationFunctionType.Exp,
                                     scale=float(1.0 / np.sqrt(HK)))
                return pp

            carry = {"pps": None}
            for qcc in range(NQ):
                qsl = slice(qcc * QC_, (qcc + 1) * QC_)
                a_qc = atp.tile([128, NHT, QC_], F32R, tag="atp")
                for hp in range(NHT):
                    # 8 accumulators [128q, 65] = (head, q-tile): 4 per
                    # head packed as manual 65-col slices of a 1-bank PSUM
                    # tile; two tiles (one per head) rotate independently in
                    # the pool so the next head-pair's PV can start as soon
                    # as THIS head's normalize reads are done
                    po0 = psO.tile([128, 512], F32, tag="psO", name="po0")
                    po1 = psO.tile([128, 512], F32, tag="psO", name="po1")

                    def acc_ap(hl, npq):
                        po = po0 if hl == 0 else po1
                        return po[:, npq * 65:npq * 65 + HK + 1]

                    def emit_pv(ktt, pp):
                        # pp chunk stationary, V moving: out accumulates the
                        # attention-natural [q, v-dims + sumexp] per head.
                        # PSUM start=True zeroes the WHOLE 2KB bank (zero
                        # region), so only the first slice written in each
                        # bank (npq == 0) carries it; the rest accumulate
                        # onto the already-zeroed bank.
                        for hl in range(2):
                            for npq in range(NPQ):
                                nc.tensor.matmul(
                                    acc_ap(hl, npq),
                                    pp[:, hl * QC_ + npq * 128:
                                       hl * QC_ + (npq + 1) * 128],
                                    v_aug[:, ktt, 2 * hp + hl, :],
                                    start=(ktt == 0) and npq == 0,
                                    stop=(ktt == NKT - 1),
                                    skip_group_check=True,
                                )

                    # two-stage software pipeline: S(kt+2) issues before
                    # PV(kt).  The first two stages of the NEXT head-pair
                    # are emitted before this pair's last PVs (cross-pair
                    # handoff) so the exp stream never waits for the 16
                    # trailing PV matmuls at a pair boundary.
                    if carry["pps"] is not None:
                        pps = carry["pps"]
                    else:
                        pps = [s_exp(qcc, hp, 0), s_exp(qcc, hp, 1)]
                    for ktt in range(2, NKT):
                        pps.append(s_exp(qcc, hp, ktt))
                        emit_pv(ktt - 2, pps[ktt - 2])
                    if hp + 1 < NHT:
                        nxt = (qcc, hp + 1)
                    elif qcc + 1 < NQ:
                        nxt = (qcc + 1, 0)
                    else:
                        nxt = None
                    if nxt is not None:
                        carry["pps"] = [s_exp(nxt[0], nxt[1], 0),
                                        s_exp(nxt[0], nxt[1], 1)]
                    else:
                        carry["pps"] = None
                    emit_pv(NKT - 2, pps[NKT - 2])
                    emit_pv(NKT - 1, pps[NKT - 1])
                    # normalize per (head, q-tile): denominator is col HK ->
                    # per-partition scalar; then PE-transpose the head pair
                    # back to the attnT layout for the output projection
                    nats = []
                    for npq in range(NPQ):
                        nat = natp.tile([128, 2 * HK], BF16, tag="natp")
                        for hl in range(2):
                            acc = acc_ap(hl, npq)
                            rz = rzp.tile([128, 1], F32, tag="rzp")
                            nc.vector.reciprocal(rz[:, :], acc[:, HK:HK + 1])
                            nc.vector.tensor_scalar_mul(
                                nat[:, hl * HK:(hl + 1) * HK],
                                acc[:, 0:HK], rz[:, :])
                        nats.append(nat)
                    tpn = psA.tile([128, NPQ, 128], BF16, tag="psA")
                    for npq in range(NPQ):
                        nc.tensor.transpose(tpn[:, npq, :], nats[npq][:, :],
                                            ident_bf[:, :])
                    nc.vector.tensor_copy(
                        a_qc[:, hp, :],
                        tpn.rearrange("p a b -> p (a b)"))

                # ---- output projection for this qc's token tiles ----
                for npq in range(NPQ):
                    n = qcc * NPQ + npq
                    for oc in range(NOC):
                        ps = psA.tile([128, QC_], F32, tag="psA")
                        for hp in range(NHT):
                            nc.tensor.matmul(
                                ps[:, :],
                                a_qc[:, hp, npq * 128:(npq + 1) * 128],
                                wo32r[:, hp, oc * QC_:(oc + 1) * QC_],
                                start=(hp == 0), stop=(hp == NHT - 1),
                            )
                        ot = outp.tile([128, QC_], F32, tag="outp")
                        nc.vector.tensor_copy(ot[:, :], ps[:, :])
                        nc.sync.dma_start(
                            out[n * 128:(n + 1) * 128, oc * QC_:(oc + 1) * QC_],
                            ot[:, :],
                        )


def build(cfg, reps=1):
    nc = bacc.Bacc("TRN2", target_bir_lowering=False, debug=False,
                   num_devices=NCORES)
    T_, D_, DH_ = cfg["T"], cfg["D"], cfg["DH"]
    ins = {
        "x": nc.dram_tensor("x", [T_, D_], F32, kind="ExternalInput").ap(),
        "wq": nc.dram_tensor("wq", [D_, DH_], F32, kind="ExternalInput").ap(),
        "wk": nc.dram_tensor("wk", [D_, DH_], F32, kind="ExternalInput").ap(),
        "wv": nc.dram_tensor("wv", [D_, DH_], F32, kind="ExternalInput").ap(),
        "wo": nc.dram_tensor("wo", [DH_, D_], F32, kind="ExternalInput").ap(),
        "bq": nc.dram_tensor("bq", [DH_], F32, kind="ExternalInput").ap(),
        "bk": nc.dram_tensor("bk", [DH_], F32, kind="ExternalInput").ap(),
        "bv": nc.dram_tensor("bv", [DH_], F32, kind="ExternalInput").ap(),
    }
    outs = {
        "out": nc.dram_tensor("out", [T_, D_], F32, kind="ExternalOutput").ap(),
    }
    with tile.TileContext(nc) as tc:
        body(tc, outs, ins, cfg, reps=reps)
    nc.compile()
    return nc


_NC_CACHE = {}


def _get_nc(reps=1):
    key = (T, D, DH, reps)
    if key not in _NC_CACHE:
        _NC_CACHE[key] = build({"T": T, "D": D, "DH": DH, "QC": QC}, reps=reps)
    return _NC_CACHE[key]


def make_in_maps(x_q, Wq, bq, Wk, bk, Wv, bv, Wo, bo):
    in_maps = []
    for c in range(NCORES):
        b, hg = divmod(c, NCORES // B)
        sl = slice(hg * DH, (hg + 1) * DH)
        in_maps.append({
            "x": np.ascontiguousarray(x_q[b], dtype=np.float32),
            "wq": np.ascontiguousarray(Wq[:, sl], dtype=np.float32),
            "wk": np.ascontiguousarray(Wk[:, sl], dtype=np.float32),
            "wv": np.ascontiguousarray(Wv[:, sl], dtype=np.float32),
            "wo": np.ascontiguousarray(Wo[sl, :], dtype=np.float32),
            "bq": np.ascontiguousarray(bq[sl], dtype=np.float32),
            "bk": np.ascontiguousarray(bk[sl], dtype=np.float32),
            "bv": np.ascontiguousarray(bv[sl], dtype=np.float32),
        })
    return in_maps


def gather(results, bo):
    ngrp = NCORES // B
    out = np.empty((B, T, D), dtype=np.float32)
    for b in range(B):
        acc = results[b * ngrp]["out"].astype(np.float32).copy()
        for hg in range(1, ngrp):
            acc += results[b * ngrp + hg]["out"]
        out[b] = acc + np.asarray(bo, dtype=np.float32)[None, :]
    return out


def kernel(x_q, Wq, bq, Wk, bk, Wv, bv, Wo, bo, _spmd_kwargs=None, _reps=1):
    from concourse.bass_utils import run_bass_kernel_spmd

    nc = _get_nc(reps=_reps)
    in_maps = make_in_maps(x_q, Wq, bq, Wk, bk, Wv, bv, Wo, bo)
    kw = _spmd_kwargs or {}
    res = run_bass_kernel_spmd(nc, in_maps, core_ids=list(range(NCORES)), **kw)
    out = gather(res.results, bo)
    kernel.last_results = res
    return out



# revision 51
# speedup vs baseline: 3.2058x; 1.0310x over previous
"""Multi-head attention Trainium2 Bass kernel.

Problem: B=2, T=2048, D=1024, H=16 heads, head dim K=64.
Sharding: 8 cores = 2 batches x 4 head-groups (4 heads each).
Each core computes q/k/v projections for its head slice, attention for its
4 heads, and a partial output projection; host sums partials over head
groups and adds the output bias.

Projections/logits run in float32r (TF32-like, full PE rate for moving
free dim >= 256); x is cast to bf16 for the PE transposes (1 cyc/row vs
2 for f32); the softmax P and V run in fp16.

Layout choices (per core):
  xT      [128p, ND, T]  x transposed, d on partitions (PE transpose of
                         the bf16-cast pages)
  qt/kt   [128p, 2, T]   Q^T/K^T: hk on partitions (head 2j at p 0:64 of
                         slab j, head 2j+1 at p 64:128) -> row-packed S
  v_aug   [128p, NT, 4, 65]  V natural (fp16) per k-tile/head + ones col
  S^T     [k=128, q]     logits transposed; exp'd to fp16 pp (ACT)
  PV      pp chunk [128k, 128q] is the STATIONARY operand, v_aug the
          moving one -> out is attention-NATURAL [128q, 65] with sumexp
          in col 64 (ones col of v_aug).  65-row moving stream per chunk
          instead of 512 -> PV stream time halves vs the S^T-moving form,
          and the softmax denominator becomes a per-partition scalar:
          normalize is reciprocal([128,1]) + tensor_scalar_mul.
  attnT   [128p, 2, T]   normalized attention transposed back via PE
                         (pair-stacked) = stationary layout for out proj
"""

from contextlib import ExitStack

import numpy as np

import concourse.bass as bass
import concourse.tile as tile
from concourse import bacc, mybir
from concourse.masks import make_identity

F32 = mybir.dt.float32
F32R = mybir.dt.float32r
BF16 = mybir.dt.bfloat16
FP16 = mybir.dt.float16

# problem config (hardcoded per contest rules)
B, T, D = 2, 2048, 1024
H, HK = 16, 64          # total heads, head dim
NCORES = 8
HPC = H // (NCORES // B)   # heads per core = 4
DH = HPC * HK              # per-core hk slice width = 256
QC = 512                   # free-dim chunk for matmuls


def body(tc, outs, ins, cfg, reps=1):
    nc = tc.nc
    T_, D_, DH_ = cfg["T"], cfg["D"], cfg["DH"]
    QC_ = cfg["QC"]
    NT = T_ // 128          # token tiles
    ND = D_ // 128          # d tiles
    NQ = T_ // QC_          # token chunks for matmul free dim
    NHT = DH_ // 128        # hk partition slabs (= head pairs)
    NKT = NT                # k-token tiles in attention

    x, wq, wk, wv, wo, bq, bk, bv = (
        ins["x"], ins["wq"], ins["wk"], ins["wv"], ins["wo"],
        ins["bq"], ins["bk"], ins["bv"],
    )
    out = outs["out"]

    with ExitStack() as ctx:
        # ---- persistent SBUF tensors ----
        xT = nc.alloc_sbuf_tensor("xT", [128, ND, T_], F32R).ap()
        qt = nc.alloc_sbuf_tensor("qt", [128, NHT, T_], F32R).ap()
        kt = nc.alloc_sbuf_tensor("kt", [128, NHT, T_], F32R).ap()
        v_aug = nc.alloc_sbuf_tensor("v_aug", [128, NKT, HPC, HK + 1], FP16).ap()
        wo32r = nc.alloc_sbuf_tensor("wo32r", [128, NHT, D_], F32R).ap()
        ident = nc.alloc_sbuf_tensor("ident", [128, 128], F32).ap()
        ident_bf = nc.alloc_sbuf_tensor("ident_bf", [128, 128], BF16).ap()
        bq_sb = nc.alloc_sbuf_tensor("bq_sb", [128, NHT], F32).ap()
        bk_sb = nc.alloc_sbuf_tensor("bk_sb", [128, NHT], F32).ap()
        bv_row = nc.alloc_sbuf_tensor("bv_row", [1, DH_], F32).ap()
        bv_bc = nc.alloc_sbuf_tensor("bv_bc", [128, DH_], F32).ap()

        # ---- pools ----
        pg4k = ctx.enter_context(tc.tile_pool(name="pg4k", bufs=3))
        wraw = ctx.enter_context(tc.tile_pool(name="wraw", bufs=1))
        w32r = ctx.enter_context(tc.tile_pool(name="w32r", bufs=3))
        ptil = ctx.enter_context(tc.tile_pool(name="ptil", bufs=4))
        atp = ctx.enter_context(tc.tile_pool(name="atp", bufs=2))
        outp = ctx.enter_context(tc.tile_pool(name="outp", bufs=2))
        natp = ctx.enter_context(tc.tile_pool(name="natp", bufs=4))
        rzp = ctx.enter_context(tc.tile_pool(name="rzp", bufs=8))
        psA = ctx.enter_context(tc.tile_pool(name="psA", bufs=2, space="PSUM"))
        psS = ctx.enter_context(tc.tile_pool(name="psS", bufs=2, space="PSUM"))
        psO = ctx.enter_context(tc.tile_pool(name="psO", bufs=2, space="PSUM"))

        make_identity(nc, ident)
        nc.vector.tensor_copy(ident_bf[:, :], ident[:, :])

        # biases (issued from gpsimd so the ACT engine stays free for exp)
        for t in range(NHT):
            nc.gpsimd.dma_start(bq_sb[:, t:t + 1],
                                bq[t * 128:(t + 1) * 128].unsqueeze(1))
            nc.gpsimd.dma_start(bk_sb[:, t:t + 1],
                                bk[t * 128:(t + 1) * 128].unsqueeze(1))
        nc.gpsimd.dma_start(bv_row[:, :], bv.unsqueeze(0))
        nc.gpsimd.partition_broadcast(bv_bc[:, :], bv_row[:, :])
        ones_f32 = nc.alloc_sbuf_tensor("ones_f32", [128, NKT * HPC], F32).ap()

        for _rep in range(reps):
            # ---- phase 0: weights load + round to f32r ----
            def load_w32r(w_dram, tag):
                wr = wraw.tile([128, ND, DH_], F32, tag="wraw")
                for dt in range(ND):
                    nc.gpsimd.dma_start(wr[:, dt, :], w_dram[dt * 128:(dt + 1) * 128, :])
                w2 = w32r.tile([128, ND, DH_], F32R, tag="w32r")
                nc.vector.tensor_copy(w2[:, :, :], wr[:, :, :])
                return w2

            wv2 = load_w32r(wv, "wv")
            # wk/wq are hoisted before the page loop so their DVE casts sit
            # ahead of the page copies in the queue -- the k/q projections
            # otherwise stall on them right after phase 1
            wk2 = load_w32r(wk, "wk")
            wq2 = load_w32r(wq, "wq")
            nc.vector.memset(ones_f32[:, :], 1.0)
            nc.vector.tensor_copy(
                v_aug[:, :, :, HK:HK + 1],
                ones_f32.rearrange("p (n h) -> p n h", h=HPC).unsqueeze(3),
            )

            # ---- phase 1: per token page: load x, cast bf16, transpose, V(n)
            # bf16 transposes run the PE at 1 cyc/row (vs 2 for f32)
            TG = 4
            for n in range(NT):
                xpg = pg4k.tile([128, D_], F32, tag="pg")
                nc.sync.dma_start(xpg[:, :], x[n * 128:(n + 1) * 128, :])
                xbf = pg4k.tile([128, D_], BF16, tag="xbf")
                # cast on ACT: idle until the first exp in a single-shot
                # run, and DVE (copies + v_aug adds) otherwise paces phase 1
                nc.scalar.copy(xbf[:, :], xpg[:, :])
                for dg in range(ND // TG):
                    tp = psA.tile([128, TG, 128], BF16, tag="psA")
                    for i in range(TG):
                        dt = dg * TG + i
                        nc.tensor.transpose(tp[:, i, :],
                                            xbf[:, dt * 128:(dt + 1) * 128],
                                            ident_bf[:, :])
                    dst = xT[:, dg * TG:(dg + 1) * TG, n * 128:(n + 1) * 128]
                    nc.vector.tensor_copy(dst, tp[:, :, :])
                # V(n): needs all d-tiles of this token page
                ps = psA.tile([128, DH_], F32, tag="psA")
                for dt in range(ND):
                    nc.tensor.matmul(
                        ps[:, :],
                        xT[:, dt, n * 128:(n + 1) * 128],
                        wv2[:, dt, :],
                        start=(dt == 0), stop=(dt == ND - 1),
                    )
                # v_aug[:, n, h, 0:HK] = ps[:, h*HK:(h+1)*HK] + bv, all h at once
                nc.vector.tensor_add(
                    v_aug[:, n, :, 0:HK],
                    ps.rearrange("p (h e) -> p h e", h=HPC),
                    bv_bc.rearrange("p (h e) -> p h e", h=HPC),
                )

            wor = wraw.tile([128, NHT, D_], F32, tag="wraw")
            for ht in range(NHT):
                nc.gpsimd.dma_start(wor[:, ht, :], wo[ht * 128:(ht + 1) * 128, :])
            nc.vector.tensor_copy(wo32r[:, :, :], wor[:, :, :])

            # ---- phase 2: q/k projections (k first: attention's blocker) ----
            def project_slab(w2, bias_sb, dst, ht):
                for qcc in range(NQ):
                    ps = psA.tile([128, QC_], F32, tag="psA")
                    for dt in range(ND):
                        nc.tensor.matmul(
                            ps[:, :],
                            w2[:, dt, ht * 128:(ht + 1) * 128],
                            xT[:, dt, qcc * QC_:(qcc + 1) * QC_],
                            start=(dt == 0), stop=(dt == ND - 1),
                        )
                    nc.vector.tensor_scalar_add(
                        dst[:, ht, qcc * QC_:(qcc + 1) * QC_], ps[:, :],
                        bias_sb[:, ht:ht + 1],
                    )

            project_slab(wk2, bk_sb, kt, 0)
            project_slab(wk2, bk_sb, kt, 1)
            project_slab(wq2, bq_sb, qt, 0)
            project_slab(wq2, bq_sb, qt, 1)

            # ---- phase 3: attention, qc-outer so projection pipelines ----
            NOC = D_ // QC_
            NPQ = QC_ // 128  # token tiles per qc chunk
            for qcc in range(NQ):
                qsl = slice(qcc * QC_, (qcc + 1) * QC_)
                a_qc = atp.tile([128, NHT, QC_], F32R, tag="atp")
                for hp in range(NHT):
                    # 8 accumulators [128q, 65] = (head, q-tile): 4 per
                    # head packed as manual 65-col slices of a 1-bank PSUM
                    # tile; two tiles (one per head) rotate independently in
                    # the pool so the next head-pair's PV can start as soon
                    # as THIS head's normalize reads are done
                    po0 = psO.tile([128, 512], F32, tag="psO", name="po0")
                    po1 = psO.tile([128, 512], F32, tag="psO", name="po1")

                    def acc_ap(hl, npq):
                        po = po0 if hl == 0 else po1
                        return po[:, npq * 65:npq * 65 + HK + 1]

                    def emit_s_exp(ktt):
                        ksl = slice(ktt * 128, (ktt + 1) * 128)
                        sp = psS.tile([128, 2 * QC_], F32, tag="psS")
                        nc.tensor.matmul(sp[:, 0:QC_], kt[0:64, hp, ksl],
                                         qt[0:64, hp, qsl],
                                         start=True, stop=True, tile_position=(0, 0))
                        nc.tensor.matmul(sp[:, QC_:2 * QC_], kt[64:128, hp, ksl],
                                         qt[64:128, hp, qsl],
                                         start=True, stop=True, tile_position=(64, 0))
                        pp = ptil.tile([128, 2 * QC_], FP16, tag="ptil")
                        nc.scalar.activation(pp[:, :], sp[:, :],
                                             mybir.ActivationFunctionType.Exp,
                                             scale=float(1.0 / np.sqrt(HK)))
                        return pp

                    def emit_pv(ktt, pp):
                        # pp chunk stationary, V moving: out accumulates the
                        # attention-natural [q, v-dims + sumexp] per head.
                        # PSUM start=True zeroes the WHOLE 2KB bank (zero
                        # region), so only the first slice written in each
                        # bank (npq == 0) carries it; the rest accumulate
                        # onto the already-zeroed bank.
                        for hl in range(2):
                            for npq in range(NPQ):
                                nc.tensor.matmul(
                                    acc_ap(hl, npq),
                                    pp[:, hl * QC_ + npq * 128:
                                       hl * QC_ + (npq + 1) * 128],
                                    v_aug[:, ktt, 2 * hp + hl, :],
                                    start=(ktt == 0) and npq == 0,
                                    stop=(ktt == NKT - 1),
                                    skip_group_check=True,
                                )

                    # two-stage software pipeline: S(kt+2) issues before
                    # PV(kt), so the PE has QK work queued while the first
                    # PV of this head-pair waits for the accumulator tile
                    pps = [emit_s_exp(0), emit_s_exp(1)]
                    for ktt in range(2, NKT):
                        pps.append(emit_s_exp(ktt))
                        emit_pv(ktt - 2, pps[ktt - 2])
                    emit_pv(NKT - 2, pps[NKT - 2])
                    emit_pv(NKT - 1, pps[NKT - 1])
                    # normalize per (head, q-tile): denominator is col HK ->
                    # per-partition scalar; then PE-transpose the head pair
                    # back to the attnT layout for the output projection
                    nats = []
                    for npq in range(NPQ):
                        nat = natp.tile([128, 2 * HK], BF16, tag="natp")
                        for hl in range(2):
                            acc = acc_ap(hl, npq)
                            rz = rzp.tile([128, 1], F32, tag="rzp")
                            nc.vector.reciprocal(rz[:, :], acc[:, HK:HK + 1])
                            nc.vector.tensor_scalar_mul(
                                nat[:, hl * HK:(hl + 1) * HK],
                                acc[:, 0:HK], rz[:, :])
                        nats.append(nat)
                    tpn = psA.tile([128, NPQ, 128], BF16, tag="psA")
                    for npq in range(NPQ):
                        nc.tensor.transpose(tpn[:, npq, :], nats[npq][:, :],
                                            ident_bf[:, :])
                    nc.vector.tensor_copy(
                        a_qc[:, hp, :],
                        tpn.rearrange("p a b -> p (a b)"))

                # ---- output projection for this qc's token tiles ----
                for npq in range(NPQ):
                    n = qcc * NPQ + npq
                    for oc in range(NOC):
                        ps = psA.tile([128, QC_], F32, tag="psA")
                        for hp in range(NHT):
                            nc.tensor.matmul(
                                ps[:, :],
                                a_qc[:, hp, npq * 128:(npq + 1) * 128],
                                wo32r[:, hp, oc * QC_:(oc + 1) * QC_],
                                start=(hp == 0), stop=(hp == NHT - 1),
                            )
                        ot = outp.tile([128, QC_], F32, tag="outp")
                        nc.vector.tensor_copy(ot[:, :], ps[:, :])
                        nc.sync.dma_start(
                            out[n * 128:(n + 1) * 128, oc * QC_:(oc + 1) * QC_],
                            ot[:, :],
                        )


def build(cfg, reps=1):
    nc = bacc.Bacc("TRN2", target_bir_lowering=False, debug=False,
                   num_devices=NCORES)
    T_, D_, DH_ = cfg["T"], cfg["D"], cfg["DH"]
    ins = {
        "x": nc.dram_tensor("x", [T_, D_], F32, kind="ExternalInput").ap(),
        "wq": nc.dram_tensor("wq", [D_, DH_], F32, kind="ExternalInput").ap(),
        "wk": nc.dram_tensor("wk", [D_, DH_], F32, kind="ExternalInput").ap(),
        "wv": nc.dram_tensor("wv", [D_, DH_], F32, kind="ExternalInput").ap(),
        "wo": nc.dram_tensor("wo", [DH_, D_], F32, kind="ExternalInput").ap(),
        "bq": nc.dram_tensor("bq", [DH_], F32, kind="ExternalInput").ap(),
        "bk": nc.dram_tensor("bk", [DH_], F32, kind="ExternalInput").ap(),
        "bv": nc.dram_tensor("bv", [DH_], F32, kind="ExternalInput").ap(),
    }
    outs = {
        "out": nc.dram_tensor("out", [T_, D_], F32, kind="ExternalOutput").ap(),
    }
    with tile.TileContext(nc) as tc:
        body(tc, outs, ins, cfg, reps=reps)
    nc.compile()
    return nc


_NC_CACHE = {}


def _get_nc(reps=1):
    key = (T, D, DH, reps)
    if key not in _NC_CACHE:
        _NC_CACHE[key] = build({"T": T, "D": D, "DH": DH, "QC": QC}, reps=reps)
    return _NC_CACHE[key]


def make_in_maps(x_q, Wq, bq, Wk, bk, Wv, bv, Wo, bo):
    in_maps = []
    for c in range(NCORES):
        b, hg = divmod(c, NCORES // B)
        sl = slice(hg * DH, (hg + 1) * DH)
        in_maps.append({
            "x": np.ascontiguousarray(x_q[b], dtype=np.float32),
            "wq": np.ascontiguousarray(Wq[:, sl], dtype=np.float32),
            "wk": np.ascontiguousarray(Wk[:, sl], dtype=np.float32),
            "wv": np.ascontiguousarray(Wv[:, sl], dtype=np.float32),
            "wo": np.ascontiguousarray(Wo[sl, :], dtype=np.float32),
            "bq": np.ascontiguousarray(bq[sl], dtype=np.float32),
            "bk": np.ascontiguousarray(bk[sl], dtype=np.float32),
            "bv": np.ascontiguousarray(bv[sl], dtype=np.float32),
        })
    return in_maps


def gather(results, bo):
    ngrp = NCORES // B
    out = np.empty((B, T, D), dtype=np.float32)
    for b in range(B):
        acc = results[b * ngrp]["out"].astype(np.float32).copy()
        for hg in range(1, ngrp):
            acc += results[b * ngrp + hg]["out"]
        out[b] = acc + np.asarray(bo, dtype=np.float32)[None, :]
    return out


def kernel(x_q, Wq, bq, Wk, bk, Wv, bv, Wo, bo, _spmd_kwargs=None, _reps=1):
    from concourse.bass_utils import run_bass_kernel_spmd

    nc = _get_nc(reps=_reps)
    in_maps = make_in_maps(x_q, Wq, bq, Wk, bk, Wv, bv, Wo, bo)
    kw = _spmd_kwargs or {}
    res = run_bass_kernel_spmd(nc, in_maps, core_ids=list(range(NCORES)), **kw)
    out = gather(res.results, bo)
    kernel.last_results = res
    return out



# revision 52
# speedup vs baseline: 3.3125x; 1.0333x over previous
"""Multi-head attention Trainium2 Bass kernel.

Problem: B=2, T=2048, D=1024, H=16 heads, head dim K=64.
Sharding: 8 cores = 2 batches x 4 head-groups (4 heads each).
Each core computes q/k/v projections for its head slice, attention for its
4 heads, and a partial output projection; host sums partials over head
groups and adds the output bias.

Projections/logits run in float32r (TF32-like, full PE rate for moving
free dim >= 256); x is cast to bf16 for the PE transposes (1 cyc/row vs
2 for f32); the softmax P and V run in fp16.

Layout choices (per core):
  xT      [128p, ND, T]  x transposed, d on partitions (PE transpose of
                         the bf16-cast pages)
  qt/kt   [128p, 2, T]   Q^T/K^T: hk on partitions (head 2j at p 0:64 of
                         slab j, head 2j+1 at p 64:128) -> row-packed S
  v_aug   [128p, NT, 4, 65]  V natural (fp16) per k-tile/head + ones col
  S^T     [k=128, q]     logits transposed; exp'd to fp16 pp (ACT)
  PV      pp chunk [128k, 128q] is the STATIONARY operand, v_aug the
          moving one -> out is attention-NATURAL [128q, 65] with sumexp
          in col 64 (ones col of v_aug).  65-row moving stream per chunk
          instead of 512 -> PV stream time halves vs the S^T-moving form,
          and the softmax denominator becomes a per-partition scalar:
          normalize is reciprocal([128,1]) + tensor_scalar_mul.
  attnT   [128p, 2, T]   normalized attention transposed back via PE
                         (pair-stacked) = stationary layout for out proj
"""

from contextlib import ExitStack

import numpy as np

import concourse.bass as bass
import concourse.tile as tile
from concourse import bacc, mybir
from concourse.masks import make_identity

F32 = mybir.dt.float32
F32R = mybir.dt.float32r
BF16 = mybir.dt.bfloat16
FP16 = mybir.dt.float16

# problem config (hardcoded per contest rules)
B, T, D = 2, 2048, 1024
H, HK = 16, 64          # total heads, head dim
NCORES = 8
HPC = H // (NCORES // B)   # heads per core = 4
DH = HPC * HK              # per-core hk slice width = 256
QC = 512                   # free-dim chunk for matmuls


def body(tc, outs, ins, cfg, reps=1):
    nc = tc.nc
    T_, D_, DH_ = cfg["T"], cfg["D"], cfg["DH"]
    QC_ = cfg["QC"]
    NT = T_ // 128          # token tiles
    ND = D_ // 128          # d tiles
    NQ = T_ // QC_          # token chunks for matmul free dim
    NHT = DH_ // 128        # hk partition slabs (= head pairs)
    NKT = NT                # k-token tiles in attention

    x, wq, wk, wv, wo, bq, bk, bv = (
        ins["x"], ins["wq"], ins["wk"], ins["wv"], ins["wo"],
        ins["bq"], ins["bk"], ins["bv"],
    )
    out = outs["out"]

    with ExitStack() as ctx:
        # ---- persistent SBUF tensors ----
        xT = nc.alloc_sbuf_tensor("xT", [128, ND, T_], F32R).ap()
        qt = nc.alloc_sbuf_tensor("qt", [128, NHT, T_], F32R).ap()
        kt = nc.alloc_sbuf_tensor("kt", [128, NHT, T_], F32R).ap()
        v_aug = nc.alloc_sbuf_tensor("v_aug", [128, NKT, HPC, HK + 1], FP16).ap()
        wo32r = nc.alloc_sbuf_tensor("wo32r", [128, NHT, D_], F32R).ap()
        ident = nc.alloc_sbuf_tensor("ident", [128, 128], F32).ap()
        ident_bf = nc.alloc_sbuf_tensor("ident_bf", [128, 128], BF16).ap()
        bq_sb = nc.alloc_sbuf_tensor("bq_sb", [128, NHT], F32).ap()
        bk_sb = nc.alloc_sbuf_tensor("bk_sb", [128, NHT], F32).ap()
        bv_row = nc.alloc_sbuf_tensor("bv_row", [1, DH_], F32).ap()
        bv_bc = nc.alloc_sbuf_tensor("bv_bc", [128, DH_], F32).ap()

        # ---- pools ----
        pg4k = ctx.enter_context(tc.tile_pool(name="pg4k", bufs=3))
        wraw = ctx.enter_context(tc.tile_pool(name="wraw", bufs=2))
        w32r = ctx.enter_context(tc.tile_pool(name="w32r", bufs=3))
        ptil = ctx.enter_context(tc.tile_pool(name="ptil", bufs=4))
        atp = ctx.enter_context(tc.tile_pool(name="atp", bufs=2))
        outp = ctx.enter_context(tc.tile_pool(name="outp", bufs=2))
        natp = ctx.enter_context(tc.tile_pool(name="natp", bufs=4))
        rzp = ctx.enter_context(tc.tile_pool(name="rzp", bufs=8))
        psA = ctx.enter_context(tc.tile_pool(name="psA", bufs=2, space="PSUM"))
        psS = ctx.enter_context(tc.tile_pool(name="psS", bufs=2, space="PSUM"))
        psO = ctx.enter_context(tc.tile_pool(name="psO", bufs=2, space="PSUM"))

        make_identity(nc, ident)
        nc.vector.tensor_copy(ident_bf[:, :], ident[:, :])

        # biases (issued from gpsimd so the ACT engine stays free for exp)
        for t in range(NHT):
            nc.gpsimd.dma_start(bq_sb[:, t:t + 1],
                                bq[t * 128:(t + 1) * 128].unsqueeze(1))
            nc.gpsimd.dma_start(bk_sb[:, t:t + 1],
                                bk[t * 128:(t + 1) * 128].unsqueeze(1))
        nc.gpsimd.dma_start(bv_row[:, :], bv.unsqueeze(0))
        nc.gpsimd.partition_broadcast(bv_bc[:, :], bv_row[:, :])
        ones_f32 = nc.alloc_sbuf_tensor("ones_f32", [128, NKT * HPC], F32).ap()

        for _rep in range(reps):
            # ---- phase 0: weights load + round to f32r ----
            def load_w32r(w_dram, tag):
                wr = wraw.tile([128, ND, DH_], F32, tag="wraw")
                for dt in range(ND):
                    nc.gpsimd.dma_start(wr[:, dt, :], w_dram[dt * 128:(dt + 1) * 128, :])
                w2 = w32r.tile([128, ND, DH_], F32R, tag="w32r")
                nc.vector.tensor_copy(w2[:, :, :], wr[:, :, :])
                return w2

            wv2 = load_w32r(wv, "wv")
            # wk/wq are hoisted before the page loop so their DVE casts sit
            # ahead of the page copies in the queue -- the k/q projections
            # otherwise stall on them right after phase 1
            wk2 = load_w32r(wk, "wk")
            wq2 = load_w32r(wq, "wq")
            nc.vector.memset(ones_f32[:, :], 1.0)
            nc.vector.tensor_copy(
                v_aug[:, :, :, HK:HK + 1],
                ones_f32.rearrange("p (n h) -> p n h", h=HPC).unsqueeze(3),
            )

            # ---- phase 1: per token page: load x, cast bf16, transpose, V(n)
            # bf16 transposes run the PE at 1 cyc/row (vs 2 for f32)
            TG = 8
            for n in range(NT):
                xpg = pg4k.tile([128, D_], F32, tag="pg")
                nc.sync.dma_start(xpg[:, :], x[n * 128:(n + 1) * 128, :])
                xbf = pg4k.tile([128, D_], BF16, tag="xbf")
                # cast on ACT: idle until the first exp in a single-shot
                # run, and DVE (copies + v_aug adds) otherwise paces phase 1
                nc.scalar.copy(xbf[:, :], xpg[:, :])
                for dg in range(ND // TG):
                    tp = psA.tile([128, TG, 128], BF16, tag="psA")
                    for i in range(TG):
                        dt = dg * TG + i
                        nc.tensor.transpose(tp[:, i, :],
                                            xbf[:, dt * 128:(dt + 1) * 128],
                                            ident_bf[:, :])
                    dst = xT[:, dg * TG:(dg + 1) * TG, n * 128:(n + 1) * 128]
                    nc.vector.tensor_copy(dst, tp[:, :, :])
                # V(n): needs all d-tiles of this token page
                ps = psA.tile([128, DH_], F32, tag="psA")
                for dt in range(ND):
                    nc.tensor.matmul(
                        ps[:, :],
                        xT[:, dt, n * 128:(n + 1) * 128],
                        wv2[:, dt, :],
                        start=(dt == 0), stop=(dt == ND - 1),
                    )
                # v_aug[:, n, h, 0:HK] = ps[:, h*HK:(h+1)*HK] + bv, all h at once
                nc.vector.tensor_add(
                    v_aug[:, n, :, 0:HK],
                    ps.rearrange("p (h e) -> p h e", h=HPC),
                    bv_bc.rearrange("p (h e) -> p h e", h=HPC),
                )

            wor = wraw.tile([128, NHT, D_], F32, tag="wraw")
            for ht in range(NHT):
                nc.gpsimd.dma_start(wor[:, ht, :], wo[ht * 128:(ht + 1) * 128, :])
            nc.vector.tensor_copy(wo32r[:, :, :], wor[:, :, :])

            # ---- phase 2: q/k projections (k first: attention's blocker) ----
            def project_slab(w2, bias_sb, dst, ht):
                for qcc in range(NQ):
                    ps = psA.tile([128, QC_], F32, tag="psA")
                    for dt in range(ND):
                        nc.tensor.matmul(
                            ps[:, :],
                            w2[:, dt, ht * 128:(ht + 1) * 128],
                            xT[:, dt, qcc * QC_:(qcc + 1) * QC_],
                            start=(dt == 0), stop=(dt == ND - 1),
                        )
                    nc.vector.tensor_scalar_add(
                        dst[:, ht, qcc * QC_:(qcc + 1) * QC_], ps[:, :],
                        bias_sb[:, ht:ht + 1],
                    )

            project_slab(wk2, bk_sb, kt, 0)
            project_slab(wk2, bk_sb, kt, 1)
            project_slab(wq2, bq_sb, qt, 0)
            project_slab(wq2, bq_sb, qt, 1)

            # ---- phase 3: attention, qc-outer so projection pipelines ----
            NOC = D_ // QC_
            NPQ = QC_ // 128  # token tiles per qc chunk
            for qcc in range(NQ):
                qsl = slice(qcc * QC_, (qcc + 1) * QC_)
                a_qc = atp.tile([128, NHT, QC_], F32R, tag="atp")
                for hp in range(NHT):
                    # 8 accumulators [128q, 65] = (head, q-tile): 4 per
                    # head packed as manual 65-col slices of a 1-bank PSUM
                    # tile; two tiles (one per head) rotate independently in
                    # the pool so the next head-pair's PV can start as soon
                    # as THIS head's normalize reads are done
                    po0 = psO.tile([128, 512], F32, tag="psO", name="po0")
                    po1 = psO.tile([128, 512], F32, tag="psO", name="po1")

                    def acc_ap(hl, npq):
                        po = po0 if hl == 0 else po1
                        return po[:, npq * 65:npq * 65 + HK + 1]

                    def emit_s_exp(ktt):
                        ksl = slice(ktt * 128, (ktt + 1) * 128)
                        sp = psS.tile([128, 2 * QC_], F32, tag="psS")
                        nc.tensor.matmul(sp[:, 0:QC_], kt[0:64, hp, ksl],
                                         qt[0:64, hp, qsl],
                                         start=True, stop=True, tile_position=(0, 0))
                        nc.tensor.matmul(sp[:, QC_:2 * QC_], kt[64:128, hp, ksl],
                                         qt[64:128, hp, qsl],
                                         start=True, stop=True, tile_position=(64, 0))
                        pp = ptil.tile([128, 2 * QC_], FP16, tag="ptil")
                        nc.scalar.activation(pp[:, :], sp[:, :],
                                             mybir.ActivationFunctionType.Exp,
                                             scale=float(1.0 / np.sqrt(HK)))
                        return pp

                    def emit_pv(ktt, pp):
                        # pp chunk stationary, V moving: out accumulates the
                        # attention-natural [q, v-dims + sumexp] per head.
                        # PSUM start=True zeroes the WHOLE 2KB bank (zero
                        # region), so only the first slice written in each
                        # bank (npq == 0) carries it; the rest accumulate
                        # onto the already-zeroed bank.
                        for hl in range(2):
                            for npq in range(NPQ):
                                nc.tensor.matmul(
                                    acc_ap(hl, npq),
                                    pp[:, hl * QC_ + npq * 128:
                                       hl * QC_ + (npq + 1) * 128],
                                    v_aug[:, ktt, 2 * hp + hl, :],
                                    start=(ktt == 0) and npq == 0,
                                    stop=(ktt == NKT - 1),
                                    skip_group_check=True,
                                )

                    # two-stage software pipeline: S(kt+2) issues before
                    # PV(kt), so the PE has QK work queued while the first
                    # PV of this head-pair waits for the accumulator tile
                    pps = [emit_s_exp(0), emit_s_exp(1)]
                    for ktt in range(2, NKT):
                        pps.append(emit_s_exp(ktt))
                        emit_pv(ktt - 2, pps[ktt - 2])
                    emit_pv(NKT - 2, pps[NKT - 2])
                    emit_pv(NKT - 1, pps[NKT - 1])
                    # normalize per (head, q-tile): denominator is col HK ->
                    # per-partition scalar; then PE-transpose the head pair
                    # back to the attnT layout for the output projection
                    nats = []
                    for npq in range(NPQ):
                        nat = natp.tile([128, 2 * HK], BF16, tag="natp")
                        for hl in range(2):
                            acc = acc_ap(hl, npq)
                            rz = rzp.tile([128, 1], F32, tag="rzp")
                            nc.vector.reciprocal(rz[:, :], acc[:, HK:HK + 1])
                            nc.vector.tensor_scalar_mul(
                                nat[:, hl * HK:(hl + 1) * HK],
                                acc[:, 0:HK], rz[:, :])
                        nats.append(nat)
                    tpn = psA.tile([128, NPQ, 128], BF16, tag="psA")
                    for npq in range(NPQ):
                        nc.tensor.transpose(tpn[:, npq, :], nats[npq][:, :],
                                            ident_bf[:, :])
                    nc.vector.tensor_copy(
                        a_qc[:, hp, :],
                        tpn.rearrange("p a b -> p (a b)"))

                # ---- output projection for this qc's token tiles ----
                for npq in range(NPQ):
                    n = qcc * NPQ + npq
                    for oc in range(NOC):
                        ps = psA.tile([128, QC_], F32, tag="psA")
                        for hp in range(NHT):
                            nc.tensor.matmul(
                                ps[:, :],
                                a_qc[:, hp, npq * 128:(npq + 1) * 128],
                                wo32r[:, hp, oc * QC_:(oc + 1) * QC_],
                                start=(hp == 0), stop=(hp == NHT - 1),
                            )
                        ot = outp.tile([128, QC_], F32, tag="outp")
                        nc.vector.tensor_copy(ot[:, :], ps[:, :])
                        nc.sync.dma_start(
                            out[n * 128:(n + 1) * 128, oc * QC_:(oc + 1) * QC_],
                            ot[:, :],
                        )


def build(cfg, reps=1):
    nc = bacc.Bacc("TRN2", target_bir_lowering=False, debug=False,
                   num_devices=NCORES)
    T_, D_, DH_ = cfg["T"], cfg["D"], cfg["DH"]
    ins = {
        "x": nc.dram_tensor("x", [T_, D_], F32, kind="ExternalInput").ap(),
        "wq": nc.dram_tensor("wq", [D_, DH_], F32, kind="ExternalInput").ap(),
        "wk": nc.dram_tensor("wk", [D_, DH_], F32, kind="ExternalInput").ap(),
        "wv": nc.dram_tensor("wv", [D_, DH_], F32, kind="ExternalInput").ap(),
        "wo": nc.dram_tensor("wo", [DH_, D_], F32, kind="ExternalInput").ap(),
        "bq": nc.dram_tensor("bq", [DH_], F32, kind="ExternalInput").ap(),
        "bk": nc.dram_tensor("bk", [DH_], F32, kind="ExternalInput").ap(),
        "bv": nc.dram_tensor("bv", [DH_], F32, kind="ExternalInput").ap(),
    }
    outs = {
        "out": nc.dram_tensor("out", [T_, D_], F32, kind="ExternalOutput").ap(),
    }
    with tile.TileContext(nc) as tc:
        body(tc, outs, ins, cfg, reps=reps)
    nc.compile()
    return nc


_NC_CACHE = {}


def _get_nc(reps=1):
    key = (T, D, DH, reps)
    if key not in _NC_CACHE:
        _NC_CACHE[key] = build({"T": T, "D": D, "DH": DH, "QC": QC}, reps=reps)
    return _NC_CACHE[key]


def make_in_maps(x_q, Wq, bq, Wk, bk, Wv, bv, Wo, bo):
    in_maps = []
    for c in range(NCORES):
        b, hg = divmod(c, NCORES // B)
        sl = slice(hg * DH, (hg + 1) * DH)
        in_maps.append({
            "x": np.ascontiguousarray(x_q[b], dtype=np.float32),
            "wq": np.ascontiguousarray(Wq[:, sl], dtype=np.float32),
            "wk": np.ascontiguousarray(Wk[:, sl], dtype=np.float32),
            "wv": np.ascontiguousarray(Wv[:, sl], dtype=np.float32),
            "wo": np.ascontiguousarray(Wo[sl, :], dtype=np.float32),
            "bq": np.ascontiguousarray(bq[sl], dtype=np.float32),
            "bk": np.ascontiguousarray(bk[sl], dtype=np.float32),
            "bv": np.ascontiguousarray(bv[sl], dtype=np.float32),
        })
    return in_maps


def gather(results, bo):
    ngrp = NCORES // B
    out = np.empty((B, T, D), dtype=np.float32)
    for b in range(B):
        acc = results[b * ngrp]["out"].astype(np.float32).copy()
        for hg in range(1, ngrp):
            acc += results[b * ngrp + hg]["out"]
        out[b] = acc + np.asarray(bo, dtype=np.float32)[None, :]
    return out


def kernel(x_q, Wq, bq, Wk, bk, Wv, bv, Wo, bo, _spmd_kwargs=None, _reps=1):
    from concourse.bass_utils import run_bass_kernel_spmd

    nc = _get_nc(reps=_reps)
    in_maps = make_in_maps(x_q, Wq, bq, Wk, bk, Wv, bv, Wo, bo)
    kw = _spmd_kwargs or {}
    res = run_bass_kernel_spmd(nc, in_maps, core_ids=list(range(NCORES)), **kw)
    out = gather(res.results, bo)
    kernel.last_results = res
    return out

